# revision 5
# baseline (speedup 1.0000x reference)
"""Trainium2 Bass kernel for nn_MaskGen: per-sample 1x1 conv (channel dot)
+ global BatchNorm2d(1) (training-mode batch stats) + LeakyReLU(0.1).

Sharding: pure data parallel over batch B=32 -> 4 batches per core on 8 cores.
Global batch-norm stats via a tiny padded [1,8] AllGather inside the kernel.

Per core (v2 design -- sf STATIONARY, feats MOVING):
  - feats shard viewed as [256, 25600] (row b*64+c), split into 2 "groups"
    of 2 batches (128 rows = 2 batches x 64 channels on partitions).
  - The baseline made feats the stationary operand (400 LDWEIGHTS+MATMUL
    pairs with N=2); on HW that serializes into ~215ns/pair of pure
    weight-load.  Here the block-diagonal sf [128, 2] is the stationary
    operand and feats chunks [128, 512] stream as the moving operand:
    100 matmuls x 512 cols ~ 21us of PE, fully hidden under the DMA.
  - mask chunks land as [2, 512] f32 in PSUM (5-bank rotation), are
    evacuated to a [4, 25600] bf16 SBUF staging tile (DVE/ACT split
    3:2), then one SBUF->SBUF DMA per group reshapes to [128, 800]
    (partition p = batch p//32, hw block 800*(p%32)) where stats,
    normalize and the store run at full 128-lane width.
  - feats tiles stream via the two HWDGE rings (sync/scalar alternating)
    in 10 x 1.31MB loads -- the DMA stream is the roofline term.
  - Stats: per-partition sum (DVE reduce) + sumsq (ACT Square accum_out)
    on the [64, 800] group slices, partition-reduced AND broadcast by a
    ones-matmul, then an 8-core AllGather of one padded 32B row.
  - Normalize: y = mask*scale + shift (ACT Identity w/ per-partition
    scale/bias for one half, DVE tensor_scalar for the other),
    LeakyReLU as max(y, 0.1*y) on DVE, two output DMAs on both rings.

Sync-capacity constraints (walrus codegen): DMA instructions carry at most
ONE semaphore wait; _split_multi_waits hoists any extras onto standalone
EventSemaphore instructions as a safety net.
"""

import os
from contextlib import ExitStack

import numpy as np

import concourse.bass as bass
import concourse.tile as tile
from concourse import library_config, mybir
from concourse.bass_utils import run_bass_kernel_spmd

N_CORES = 8
B, C, H, W = 32, 64, 160, 160
HW = H * W                # 25600
BPC = B // N_CORES        # 4 batches per core
NG = BPC // 2             # 2 groups (pairs of batches) per core
ROWS = BPC * C            # 256 feats rows per core
N_TOT = B * HW            # 819200 elements in the batchnorm stats
MMW = 512                 # moving-operand width per matmul (1 PSUM bank f32)
TILE_W = 5120             # feats DMA tile width (1.31 MB per load)
NLOAD = HW // TILE_W      # 5 loads per group
MM_PER_LOAD = TILE_W // MMW  # 10 matmuls per loaded tile
CPG = HW // MMW           # 50 mask chunks per group
RW = BPC * (HW // 128)    # 800 = reshaped-stage cols ([128, 800])
EPS = 1e-5
SLOPE = 0.1

F32 = mybir.dt.float32
IN_DT = mybir.dt.bfloat16
IN_DT_NP = np.dtype(mybir.dt.np(mybir.dt.bfloat16))


def _body(ctx: ExitStack, tc: "tile.TileContext", feats, sf, bnwb, out):
    nc = tc.nc
    AF = mybir.ActivationFunctionType
    ALU = mybir.AluOpType

    singles = ctx.enter_context(tc.tile_pool(name="singles", bufs=1))
    # one slot per feats tile: no slot reuse -> feats DMAs carry no WAR wait
    ftp = ctx.enter_context(tc.tile_pool(name="ftp", bufs=NG * NLOAD))
    psc = ctx.enter_context(tc.tile_pool(name="psc", bufs=5, space="PSUM"))
    pss = ctx.enter_context(tc.tile_pool(name="pss", bufs=1, space="PSUM"))
    dram = ctx.enter_context(tc.tile_pool(name="dram", bufs=1, space="DRAM"))

    # --- block-diagonal sf weights (host-precomputed): col 2g+r holds
    #     sf[2g+r,:] in rows 64r:64r+64, zeros elsewhere.
    w_sb = singles.tile([128, 2 * NG], IN_DT)
    nc.sync.dma_start(out=w_sb, in_=sf)

    # ones for the partition-reduce + broadcast matmul
    ones_sb = singles.tile([128, 128], F32)
    nc.vector.memset(ones_sb, 1.0)

    # bn weight+bias broadcast to all partitions: [128, 2] = [w, b]
    wbb = singles.tile([128, 2], F32, tag="wbb")
    nc.scalar.dma_start(out=wbb, in_=bnwb.to_broadcast([128, 2]))

    eps_sb = singles.tile([128, 1], F32, tag="eps_sb")
    nc.vector.memset(eps_sb, EPS)

    # padded 32-byte collective payload row: [sum, sumsq, 0...]
    cc_src = singles.tile([1, 8], F32, tag="cc_src")
    nc.vector.memset(cc_src, 0.0)

    # mask staging: group g's 2 batch rows live at partitions 32g/32g+1
    # (engine APs must start at partition 0/32/64/96), reshaped to [128, 800]
    mstage = singles.tile([64, HW], IN_DT, tag="mstage")
    rstage = singles.tile([128, RW], IN_DT, tag="rstage")
    sqwork = singles.tile([128, RW], IN_DT, tag="sqwork")
    pp2 = singles.tile([128, 2], F32, tag="pp2")  # [sum, sumsq] per partition

    # PE warm-up dummies: absorb the w_sb-DMA and ones-memset waits into
    # PE's vector clock so no later matmul needs a second wait slot.
    warm_ps = pss.tile([128, 1], F32, tag="warm")
    nc.tensor.matmul(out=warm_ps[: 2 * NG, :], lhsT=w_sb, rhs=w_sb[:, 0:1],
                     start=True, stop=True)
    nc.tensor.matmul(out=warm_ps, lhsT=ones_sb, rhs=ones_sb[:, 0:1],
                     start=True, stop=True)

    # --- channel-dot matmuls: sf stationary, feats chunks stream through
    for g in range(NG):
        lw = w_sb[:, 2 * g : 2 * g + 2]
        for l in range(NLOAD):
            ft = ftp.tile([128, TILE_W], IN_DT, tag="ft")
            eng = nc.sync if (g * NLOAD + l) % 2 == 0 else nc.scalar
            eng.dma_start(
                out=ft,
                in_=feats[128 * g : 128 * (g + 1), TILE_W * l : TILE_W * (l + 1)],
            )
            for m in range(MM_PER_LOAD):
                ch = MM_PER_LOAD * l + m    # chunk index within group
                cp = psc.tile([2, MMW], F32, tag="chunk")
                nc.tensor.matmul(
                    out=cp,
                    lhsT=lw,
                    rhs=ft[:, MMW * m : MMW * (m + 1)],
                    start=True,
                    stop=True,
                )
                dst = mstage[32 * g : 32 * g + 2, MMW * ch : MMW * (ch + 1)]
                if ch % 5 < 3:
                    nc.vector.tensor_copy(out=dst, in_=cp)
                else:
                    nc.scalar.activation(out=dst, in_=cp, func=AF.Identity)
        # group reshape: [2, 25600] -> rstage[64g:64g+64, :] = [64, 800]
        nc.sync.dma_start(
            out=rstage[64 * g : 64 * (g + 1), :],
            in_=mstage[32 * g : 32 * g + 2].rearrange(
                "p (s c) -> p s c", s=32, c=RW
            ),
        )
        # group stats: sum on DVE, sumsq on ACT (Square + accum_out)
        nc.vector.tensor_reduce(
            out=pp2[64 * g : 64 * (g + 1), 0:1],
            in_=rstage[64 * g : 64 * (g + 1), :],
            axis=mybir.AxisListType.X,
            op=ALU.add,
        )
        nc.scalar.activation(
            out=sqwork[64 * g : 64 * (g + 1), :],
            in_=rstage[64 * g : 64 * (g + 1), :],
            func=AF.Square,
            accum_out=pp2[64 * g : 64 * (g + 1), 1:2],
        )

    # partition-reduce AND broadcast: stats_ps[m, j] = sum_p pp2[p, j]
    stats_ps = pss.tile([128, 2], F32, tag="stats")
    nc.tensor.matmul(out=stats_ps, lhsT=ones_sb, rhs=pp2, start=True, stop=True)
    nc.vector.tensor_copy(out=cc_src[:, 0:2], in_=stats_ps[0:1, :])

    # --- AllGather one padded 32B row per core (cheaper than AllReduce:
    # plain copy chunks, no CCE reduce reads); the cross-core sum happens
    # on-core below.
    cc_in = dram.tile([1, 8], F32, tag="cc_in")
    cc_out = dram.tile([1, 8 * N_CORES], F32, tag="cc_out")
    nc.scalar.dma_start(out=cc_in[:], in_=cc_src)
    nc.gpsimd.collective_compute(
        "AllGather",
        mybir.AluOpType.bypass,
        replica_groups=[list(range(N_CORES))],
        ins=[cc_in.opt()],
        outs=[cc_out.opt()],
    )
    # gathered rows back as one 256B row, partition-broadcast via a K=1
    # matmul (avoids the slower DRE-replication DMA descriptor pattern).
    allred_sb = singles.tile([1, 8 * N_CORES], F32, tag="allred_sb")
    nc.scalar.dma_start(out=allred_sb, in_=cc_out[:])
    stats_bc = pss.tile([128, 8 * N_CORES], F32, tag="stats_bc")
    nc.tensor.matmul(out=stats_bc, lhsT=ones_sb[0:1, :], rhs=allred_sb,
                     start=True, stop=True)
    # sum the 8 per-rank [sum, sumsq] pairs: view [128, (rank, col)] as
    # [128, col, rank] and reduce the innermost rank axis.
    totals = singles.tile([128, 2], F32, tag="totals")
    nc.vector.tensor_reduce(
        out=totals,
        in_=stats_bc[:, 0 : 8 * N_CORES].rearrange(
            "p (r c) -> p c r", r=N_CORES, c=8
        )[:, 0:2, :],
        axis=mybir.AxisListType.X,
        op=ALU.add,
    )

    # --- scalar math, replicated across partitions ([128,1] tiles)
    me2 = singles.tile([128, 2], F32, tag="me2")   # [mean, E[x^2]]
    nc.vector.tensor_scalar_mul(out=me2, in0=totals, scalar1=1.0 / N_TOT)
    msq = singles.tile([128, 1], F32, tag="msq")
    nc.vector.tensor_mul(out=msq, in0=me2[:, 0:1], in1=me2[:, 0:1])
    var = singles.tile([128, 1], F32, tag="var")
    nc.vector.tensor_sub(out=var, in0=me2[:, 1:2], in1=msq)
    std = singles.tile([128, 1], F32, tag="std")
    nc.scalar.activation(out=std, in_=var, func=AF.Sqrt, bias=eps_sb)
    inv = singles.tile([128, 1], F32, tag="inv")
    nc.vector.reciprocal(out=inv, in_=std)
    scl = singles.tile([128, 1], F32, tag="scl")
    nc.vector.tensor_mul(out=scl, in0=inv, in1=wbb[:, 0:1])
    msc = singles.tile([128, 1], F32, tag="msc")
    nc.vector.tensor_mul(out=msc, in0=me2[:, 0:1], in1=scl)
    shf = singles.tile([128, 1], F32, tag="shf")
    nc.vector.tensor_sub(out=shf, in0=wbb[:, 1:2], in1=msc)

    # --- normalize + LeakyReLU + store from rstage [128, 800]
    # (host un-permutes: partition p = batch p//32, hw block 800*(p%32))
    hwl = RW // 2
    y0 = singles.tile([128, hwl], F32, tag="y0")
    nc.scalar.activation(out=y0, in_=rstage[:, 0:hwl], func=AF.Identity,
                         bias=shf, scale=scl)
    o0 = singles.tile([128, hwl], IN_DT, tag="o0")
    nc.vector.scalar_tensor_tensor(
        out=o0, in0=y0, scalar=SLOPE, in1=y0, op0=ALU.mult, op1=ALU.max
    )
    nc.sync.dma_start(out=out[:, 0:hwl], in_=o0)

    y1 = singles.tile([128, hwl], F32, tag="y1")
    nc.vector.tensor_scalar(
        out=y1, in0=rstage[:, hwl:RW], scalar1=scl, scalar2=shf,
        op0=ALU.mult, op1=ALU.add,
    )
    o1 = singles.tile([128, hwl], IN_DT, tag="o1")
    nc.vector.scalar_tensor_tensor(
        out=o1, in0=y1, scalar=SLOPE, in1=y1, op0=ALU.mult, op1=ALU.max
    )
    # second store on the ACT HWDGE ring so both output DMAs dispatch in
    # parallel with the first on the SP ring.
    nc.scalar.dma_start(out=out[:, hwl:RW], in_=o1)


def _split_multi_waits(nc):
    """walrus codegen accepts one semaphore wait per instruction (each ISA
    struct embeds a single EVENTS slot).  Tile's scheduler attaches several;
    hoist all but the last onto standalone EventSemaphore instructions on the
    same engine, immediately before the original instruction."""
    n = 0
    for fn in nc.m.functions:
        for bb in fn.blocks:
            insts = list(bb.instructions)
            if not any(
                i.sync_info is not None and len(i.sync_info.on_wait) > 1
                for i in insts
            ):
                continue
            new_insts = []
            for inst in insts:
                si = inst.sync_info
                if si is not None and len(si.on_wait) > 1:
                    waits = list(si.on_wait)
                    for w in waits[:-1]:
                        n += 1
                        ev = mybir.InstEventSemaphore(
                            name=f"{inst.name}-sw{n}",
                            ins=[],
                            outs=[],
                            sync_info=mybir.SyncInfo(on_wait=[w], on_update=[]),
                        )
                        ev.engine = inst.engine
                        nc.register_instruction(ev, overwrite=True)
                        new_insts.append(ev)
                    si.on_wait = [waits[-1]]
                new_insts.append(inst)
            bb.instructions = new_insts
    return n


def build_nc():
    nc = bass.Bass(num_devices=N_CORES)
    feats = nc.declare_dram_parameter("feats", [ROWS, HW], IN_DT, isOutput=False)
    sf = nc.declare_dram_parameter("sf", [128, 2 * NG], IN_DT, isOutput=False)
    bnwb = nc.declare_dram_parameter("bn_wb", [1, 2], F32, isOutput=False)
    out = nc.declare_dram_parameter("out", [128, RW], IN_DT, isOutput=True)
    with tile.TileContext(nc, num_cores=N_CORES) as tc:
        with ExitStack() as ctx:
            _body(ctx, tc, feats[:], sf[:], bnwb[:], out[:])
    _split_multi_waits(nc)
    return nc


def make_in_maps(sf, feats, bn_weight, bn_bias):
    sf = np.asarray(sf)
    feats = np.asarray(feats)
    bnwb = np.array(
        [[np.float32(np.asarray(bn_weight).reshape(-1)[0]),
          np.float32(np.asarray(bn_bias).reshape(-1)[0])]],
        dtype=np.float32,
    )
    sf2 = np.ascontiguousarray(sf.reshape(B, C)).astype(IN_DT_NP)
    in_maps = []
    for k in range(N_CORES):
        fshard = np.ascontiguousarray(
            feats[BPC * k : BPC * (k + 1)].reshape(ROWS, HW)
        ).astype(IN_DT_NP)
        wmat = np.zeros((128, 2 * NG), dtype=IN_DT_NP)
        for g in range(NG):
            for r in range(2):
                wmat[64 * r : 64 * r + 64, 2 * g + r] = sf2[BPC * k + 2 * g + r]
        in_maps.append(
            {
                "feats": fshard,
                "sf": wmat,
                "bn_wb": bnwb,
            }
        )
    return in_maps


_NC_CACHE = {}


def get_nc():
    if "nc" not in _NC_CACHE:
        _NC_CACHE["nc"] = build_nc()
    return _NC_CACHE["nc"]


def assemble(results):
    parts = []
    for r in results:
        a = np.asarray(r["out"], dtype=np.float32).reshape(128, RW)
        # partition p = (batch p//32, hw block 800*(p%32)) -> [BPC, HW]
        parts.append(a.reshape(BPC, 32 * RW))
    return np.concatenate(parts, axis=0).reshape(B, 1, H, W).astype(np.float32)


def kernel(sf, feats, bn_weight, bn_bias):
    nc = get_nc()
    in_maps = make_in_maps(sf, feats, bn_weight, bn_bias)
    res = run_bass_kernel_spmd(nc, in_maps, list(range(N_CORES)))
    return assemble(res.results)


# revision 15
# speedup vs baseline: 1.1136x; 1.1136x over previous
"""Trainium2 Bass kernel for nn_MaskGen: per-sample 1x1 conv (channel dot)
+ global BatchNorm2d(1) (training-mode batch stats) + LeakyReLU(0.1).

Sharding: pure data parallel over batch B=32 -> 4 batches per core on 8 cores.
Global batch-norm stats via a tiny padded [1,8] AllGather inside the kernel.

Per core (v3 design -- sf STATIONARY, feats MOVING, bank-packed PSUM):
  - feats shard viewed as [256, 25600] (row b*64+c), split into 2 "groups"
    of 2 batches (128 rows = 2 batches x 64 channels on partitions).
  - The block-diagonal sf [128, 2] is the stationary operand and feats
    chunks [128, 512] stream as the moving operand: 100 matmuls x 512
    cols ~ 21us of PE, hidden under the ~37us feats DMA stream.
  - PSUM bank packing: 4 consecutive hw-chunks (slot k = j%4) of a group
    land in ONE bank at partition offsets 32k (tile_position col-groups),
    so one engine copy evacuates 4 chunks ([128, 512] costs the same as
    [2, 512] -- engine time scales with free size, not partitions).
    26 evacuations (13 fills x 2 groups) alternate DVE/ACT.
  - mstage[32k+r, 512*(2t+g) + o] = mask[batch 2g+r, hw 512*(4t+k)+o].
    Per-group reshape DMAs (SBUF->SBUF) regather into a partition-dense
    rstage[64g+32r+8k+oh, 64t+ol] (o = 64*oh+ol), where stats, normalize
    and the store run at full 128-lane width.  The ragged 13th fill
    (chunks 48,49 -> k<2 only) goes in a second small DMA; the unwritten
    rstage cells are memset to 0 up front so stats stay exact.
  - feats tiles stream via the two HWDGE rings (sync/scalar alternating)
    in 10 x 1.31MB loads -- the DMA stream is the roofline term.
  - Stats: per-partition sum (DVE reduce) + sumsq (ACT Square accum_out)
    on rstage, partition-reduced AND broadcast by a ones-matmul, then an
    8-core AllGather of one padded 32B row.
  - Normalize: y = mask*scale + shift (ACT Identity w/ per-partition
    scale/bias for one half, DVE tensor_scalar for the other),
    LeakyReLU as max(y, 0.1*y) on DVE, two output DMAs on both rings.

Sync-capacity constraints (walrus codegen): DMA instructions carry at most
ONE semaphore wait; _split_multi_waits hoists any extras onto standalone
EventSemaphore instructions as a safety net.
"""

import os
from contextlib import ExitStack

import numpy as np

import concourse.bass as bass
import concourse.tile as tile
from concourse import library_config, mybir
from concourse.bass_utils import run_bass_kernel_spmd

N_CORES = 8
B, C, H, W = 32, 64, 160, 160
HW = H * W                # 25600
BPC = B // N_CORES        # 4 batches per core
NG = BPC // 2             # 2 groups (pairs of batches) per core
ROWS = BPC * C            # 256 feats rows per core
N_TOT = B * HW            # 819200 elements in the batchnorm stats
MMW = 512                 # moving-operand width per matmul
TILE_W = 5120             # feats DMA tile width (1.31 MB per load)
NLOAD = HW // TILE_W      # 5 loads per group
MM_PER_LOAD = TILE_W // MMW  # 10 matmuls per loaded tile
CPG = HW // MMW           # 50 mask chunks per group
NFILL = (CPG + 3) // 4    # 13 bank fills per group (fill 12 has k=0,1 only)
NFP = 16                  # padded fills (t addressing), fills 13..15 unused
MSW = 2 * NFP * MMW       # 16384 mstage cols (block index NFP*g + t)
RW = 2 * MMW              # 1024 rstage cols (col = 512*(t%2) + o)
EPS = 1e-5
SLOPE = 0.1

F32 = mybir.dt.float32
IN_DT = mybir.dt.bfloat16
IN_DT_NP = np.dtype(mybir.dt.np(mybir.dt.bfloat16))


def _body(ctx: ExitStack, tc: "tile.TileContext", feats, sf, bnwb, out):
    nc = tc.nc
    AF = mybir.ActivationFunctionType
    ALU = mybir.AluOpType

    singles = ctx.enter_context(tc.tile_pool(name="singles", bufs=1))
    # one slot per feats tile: no slot reuse -> feats DMAs carry no WAR wait
    ftp = ctx.enter_context(tc.tile_pool(name="ftp", bufs=NG * NLOAD))
    psc = ctx.enter_context(tc.tile_pool(name="psc", bufs=5, space="PSUM"))
    pss = ctx.enter_context(tc.tile_pool(name="pss", bufs=1, space="PSUM"))
    dram = ctx.enter_context(tc.tile_pool(name="dram", bufs=1, space="DRAM"))

    # --- block-diagonal sf weights (host-precomputed): col 2g+r holds
    #     sf[2g+r,:] in rows 64r:64r+64, zeros elsewhere.
    w_sb = singles.tile([128, 2 * NG], IN_DT)
    nc.sync.dma_start(out=w_sb, in_=sf)

    # ones for the partition-reduce + broadcast matmul
    ones_sb = singles.tile([128, 128], F32)
    nc.vector.memset(ones_sb, 1.0)

    # bn weight+bias broadcast to all partitions: [128, 2] = [w, b]
    wbb = singles.tile([128, 2], F32, tag="wbb")
    nc.scalar.dma_start(out=wbb, in_=bnwb.to_broadcast([128, 2]))

    eps_sb = singles.tile([128, 1], F32, tag="eps_sb")
    nc.vector.memset(eps_sb, EPS)

    # padded 32-byte collective payload row: [sum, sumsq, 0...]
    cc_src = singles.tile([1, 8], F32, tag="cc_src")
    nc.vector.memset(cc_src, 0.0)

    # mask staging (bank-order) and partition-dense restage
    mstage = singles.tile([128, MSW], IN_DT, tag="mstage")
    rstage = singles.tile([128, RW], IN_DT, tag="rstage")
    sqwork = singles.tile([128, RW], IN_DT, tag="sqwork")
    pp2 = singles.tile([128, 2], F32, tag="pp2")  # [sum, sumsq] per partition

    # zero the padded mstage fill blocks (t=12 slots k>=2 via the partial
    # t=12 evacuation, and t=13..15 entirely): the reshape DMA copies them
    # into rstage, where they must read as 0 so the stats stay exact.
    for g in range(NG):
        nc.vector.memset(
            mstage[:, MMW * (NFP * g + 12) : MMW * (NFP * g + NFP)], 0.0
        )

    # PE warm-up dummies: absorb the w_sb-DMA and ones-memset waits into
    # PE's vector clock so no later matmul needs a second wait slot.
    warm_ps = pss.tile([128, 1], F32, tag="warm")
    nc.tensor.matmul(out=warm_ps[: 2 * NG, :], lhsT=w_sb, rhs=w_sb[:, 0:1],
                     start=True, stop=True)
    nc.tensor.matmul(out=warm_ps, lhsT=ones_sb, rhs=ones_sb[:, 0:1],
                     start=True, stop=True)

    # --- channel-dot matmuls: sf stationary, feats chunks stream through.
    # 4 chunks share a PSUM bank at partition offsets 32k; one [128, 512]
    # copy evacuates the whole bank into mstage block T = 2t + g.
    nev = 0
    for g in range(NG):
        lw = w_sb[:, 2 * g : 2 * g + 2]
        bank = None
        for l in range(NLOAD):
            ft = ftp.tile([128, TILE_W], IN_DT, tag="ft")
            eng = nc.sync if (g * NLOAD + l) % 2 == 0 else nc.scalar
            eng.dma_start(
                out=ft,
                in_=feats[128 * g : 128 * (g + 1), TILE_W * l : TILE_W * (l + 1)],
            )
            for m in range(MM_PER_LOAD):
                j = MM_PER_LOAD * l + m     # chunk index within group
                k = j % 4                   # bank slot -> partition 32k
                if k == 0:
                    bank = psc.tile([128, MMW], F32, tag="bank")
                nc.tensor.matmul(
                    out=bank[32 * k : 32 * k + 2, :],
                    lhsT=lw,
                    rhs=ft[:, MMW * m : MMW * (m + 1)],
                    start=True,
                    stop=True,
                    tile_position=(0, 32 * k),
                )
                if k == 3 or j == CPG - 1:
                    t = j // 4
                    co = MMW * (NFP * g + t)
                    # the ragged fill (t=12, chunks 48/49 at k=0,1) copies
                    # only partitions 0..63, preserving the memset zeros at
                    # rows 64+ that the reshape DMA reads for k>=2.
                    rows = 128 if k == 3 else 64
                    dst = mstage[0:rows, co : co + MMW]
                    if nev % 2 == 0:
                        nc.vector.tensor_copy(out=dst, in_=bank[0:rows, :])
                    else:
                        nc.scalar.activation(out=dst, in_=bank[0:rows, :],
                                             func=AF.Identity)
                    nev += 1
        # group reshape (SBUF->SBUF), one 3-dim DMA per group:
        #   mstage[32k+r, 512*(16g+t)+o] -> rstage[64g+32r+8k+(t//2),
        #   512*(t%2)+o].  Source cols for a (k, r) row are one contiguous
        #   8192-elem run; dest is a plain [64, 1024] partition-dense slice
        #   whose row index 32r+8k+th nests exactly as the (r, k, t-major)
        #   source walk.
        msv = mstage.rearrange(
            "(k r32) (g2 t o) -> r32 k g2 t o",
            k=4, r32=32, g2=2, t=NFP, o=MMW,
        )
        for r in range(2):
            eng = nc.sync if r == 0 else nc.scalar
            eng.dma_start(
                out=rstage[64 * g + 32 * r : 64 * g + 32 * r + 32, :],
                in_=msv[r, :, g, :, :],
            )
        # group stats: sum on DVE, sumsq on ACT (Square + accum_out)
        nc.vector.tensor_reduce(
            out=pp2[64 * g : 64 * (g + 1), 0:1],
            in_=rstage[64 * g : 64 * (g + 1), :],
            axis=mybir.AxisListType.X,
            op=ALU.add,
        )
        nc.scalar.activation(
            out=sqwork[64 * g : 64 * (g + 1), :],
            in_=rstage[64 * g : 64 * (g + 1), :],
            func=AF.Square,
            accum_out=pp2[64 * g : 64 * (g + 1), 1:2],
        )

    # partition-reduce AND broadcast: stats_ps[m, j] = sum_p pp2[p, j]
    stats_ps = pss.tile([128, 2], F32, tag="stats")
    nc.tensor.matmul(out=stats_ps, lhsT=ones_sb, rhs=pp2, start=True, stop=True)
    nc.vector.tensor_copy(out=cc_src[:, 0:2], in_=stats_ps[0:1, :])

    # --- AllGather one padded 32B row per core (cheaper than AllReduce:
    # plain copy chunks, no CCE reduce reads); the cross-core sum happens
    # on-core below.
    cc_in = dram.tile([1, 8], F32, tag="cc_in")
    cc_out = dram.tile([1, 8 * N_CORES], F32, tag="cc_out")
    nc.scalar.dma_start(out=cc_in[:], in_=cc_src)
    nc.gpsimd.collective_compute(
        "AllGather",
        mybir.AluOpType.bypass,
        replica_groups=[list(range(N_CORES))],
        ins=[cc_in.opt()],
        outs=[cc_out.opt()],
    )
    # gathered rows back as one 256B row, partition-broadcast via a K=1
    # matmul (avoids the slower DRE-replication DMA descriptor pattern).
    allred_sb = singles.tile([1, 8 * N_CORES], F32, tag="allred_sb")
    nc.scalar.dma_start(out=allred_sb, in_=cc_out[:])
    stats_bc = pss.tile([128, 8 * N_CORES], F32, tag="stats_bc")
    nc.tensor.matmul(out=stats_bc, lhsT=ones_sb[0:1, :], rhs=allred_sb,
                     start=True, stop=True)
    # sum the 8 per-rank [sum, sumsq] pairs: view [128, (rank, col)] as
    # [128, col, rank] and reduce the innermost rank axis.
    totals = singles.tile([128, 2], F32, tag="totals")
    nc.vector.tensor_reduce(
        out=totals,
        in_=stats_bc[:, 0 : 8 * N_CORES].rearrange(
            "p (r c) -> p c r", r=N_CORES, c=8
        )[:, 0:2, :],
        axis=mybir.AxisListType.X,
        op=ALU.add,
    )

    # --- scalar math, replicated across partitions ([128,1] tiles)
    me2 = singles.tile([128, 2], F32, tag="me2")   # [mean, E[x^2]]
    nc.vector.tensor_scalar_mul(out=me2, in0=totals, scalar1=1.0 / N_TOT)
    msq = singles.tile([128, 1], F32, tag="msq")
    nc.vector.tensor_mul(out=msq, in0=me2[:, 0:1], in1=me2[:, 0:1])
    var = singles.tile([128, 1], F32, tag="var")
    nc.vector.tensor_sub(out=var, in0=me2[:, 1:2], in1=msq)
    std = singles.tile([128, 1], F32, tag="std")
    nc.scalar.activation(out=std, in_=var, func=AF.Sqrt, bias=eps_sb)
    inv = singles.tile([128, 1], F32, tag="inv")
    nc.vector.reciprocal(out=inv, in_=std)
    scl = singles.tile([128, 1], F32, tag="scl")
    nc.vector.tensor_mul(out=scl, in0=inv, in1=wbb[:, 0:1])
    msc = singles.tile([128, 1], F32, tag="msc")
    nc.vector.tensor_mul(out=msc, in0=me2[:, 0:1], in1=scl)
    shf = singles.tile([128, 1], F32, tag="shf")
    nc.vector.tensor_sub(out=shf, in0=wbb[:, 1:2], in1=msc)

    # --- normalize + LeakyReLU + store from rstage [128, 1024]
    # (host un-permutes; cells from padded fills are dropped there)
    hwl = RW // 2
    y0 = singles.tile([128, hwl], F32, tag="y0")
    nc.scalar.activation(out=y0, in_=rstage[:, 0:hwl], func=AF.Identity,
                         bias=shf, scale=scl)
    o0 = singles.tile([128, hwl], IN_DT, tag="o0")
    nc.vector.scalar_tensor_tensor(
        out=o0, in0=y0, scalar=SLOPE, in1=y0, op0=ALU.mult, op1=ALU.max
    )
    nc.sync.dma_start(out=out[:, 0:hwl], in_=o0)

    y1 = singles.tile([128, hwl], F32, tag="y1")
    nc.vector.tensor_scalar(
        out=y1, in0=rstage[:, hwl:RW], scalar1=scl, scalar2=shf,
        op0=ALU.mult, op1=ALU.add,
    )
    o1 = singles.tile([128, hwl], IN_DT, tag="o1")
    nc.vector.scalar_tensor_tensor(
        out=o1, in0=y1, scalar=SLOPE, in1=y1, op0=ALU.mult, op1=ALU.max
    )
    # second store on the ACT HWDGE ring so both output DMAs dispatch in
    # parallel with the first on the SP ring.
    nc.scalar.dma_start(out=out[:, hwl:RW], in_=o1)


def _split_multi_waits(nc):
    """walrus codegen accepts one semaphore wait per instruction (each ISA
    struct embeds a single EVENTS slot).  Tile's scheduler attaches several;
    hoist all but the last onto standalone EventSemaphore instructions on the
    same engine, immediately before the original instruction."""
    n = 0
    for fn in nc.m.functions:
        for bb in fn.blocks:
            insts = list(bb.instructions)
            if not any(
                i.sync_info is not None and len(i.sync_info.on_wait) > 1
                for i in insts
            ):
                continue
            new_insts = []
            for inst in insts:
                si = inst.sync_info
                if si is not None and len(si.on_wait) > 1:
                    waits = list(si.on_wait)
                    for w in waits[:-1]:
                        n += 1
                        ev = mybir.InstEventSemaphore(
                            name=f"{inst.name}-sw{n}",
                            ins=[],
                            outs=[],
                            sync_info=mybir.SyncInfo(on_wait=[w], on_update=[]),
                        )
                        ev.engine = inst.engine
                        nc.register_instruction(ev, overwrite=True)
                        new_insts.append(ev)
                    si.on_wait = [waits[-1]]
                new_insts.append(inst)
            bb.instructions = new_insts
    return n


def build_nc():
    nc = bass.Bass(num_devices=N_CORES)
    feats = nc.declare_dram_parameter("feats", [ROWS, HW], IN_DT, isOutput=False)
    sf = nc.declare_dram_parameter("sf", [128, 2 * NG], IN_DT, isOutput=False)
    bnwb = nc.declare_dram_parameter("bn_wb", [1, 2], F32, isOutput=False)
    out = nc.declare_dram_parameter("out", [128, RW], IN_DT, isOutput=True)
    with tile.TileContext(nc, num_cores=N_CORES) as tc:
        with ExitStack() as ctx:
            _body(ctx, tc, feats[:], sf[:], bnwb[:], out[:])
    _split_multi_waits(nc)
    return nc


def make_in_maps(sf, feats, bn_weight, bn_bias):
    sf = np.asarray(sf)
    feats = np.asarray(feats)
    bnwb = np.array(
        [[np.float32(np.asarray(bn_weight).reshape(-1)[0]),
          np.float32(np.asarray(bn_bias).reshape(-1)[0])]],
        dtype=np.float32,
    )
    sf2 = np.ascontiguousarray(sf.reshape(B, C)).astype(IN_DT_NP)
    in_maps = []
    for k in range(N_CORES):
        fshard = np.ascontiguousarray(
            feats[BPC * k : BPC * (k + 1)].reshape(ROWS, HW)
        ).astype(IN_DT_NP)
        wmat = np.zeros((128, 2 * NG), dtype=IN_DT_NP)
        for g in range(NG):
            for r in range(2):
                wmat[64 * r : 64 * r + 64, 2 * g + r] = sf2[BPC * k + 2 * g + r]
        in_maps.append(
            {
                "feats": fshard,
                "sf": wmat,
                "bn_wb": bnwb,
            }
        )
    return in_maps


_NC_CACHE = {}


def get_nc():
    if "nc" not in _NC_CACHE:
        _NC_CACHE["nc"] = build_nc()
    return _NC_CACHE["nc"]


def assemble(results):
    parts = []
    for r in results:
        a = np.asarray(r["out"], dtype=np.float32).reshape(2, 2, 4, 8, 2, MMW)
        # [g, r, k, th, tl, o] -> [b=2g+r, t=2th+tl, k, o] -> hw=512*(4t+k)+o
        yv = a.transpose(0, 1, 3, 4, 2, 5).reshape(BPC, NFP, 4, MMW)
        parts.append(yv.reshape(BPC, NFP * 4 * MMW)[:, :HW])
    return np.concatenate(parts, axis=0).reshape(B, 1, H, W).astype(np.float32)


def kernel(sf, feats, bn_weight, bn_bias):
    nc = get_nc()
    in_maps = make_in_maps(sf, feats, bn_weight, bn_bias)
    res = run_bass_kernel_spmd(nc, in_maps, list(range(N_CORES)))
    return assemble(res.results)
